# revision 3
# baseline (speedup 1.0000x reference)
"""Trainium2 Bass kernel for GQA sliding-window attention with logit soft-cap.

Problem: B=2, T=2048, D=3584, N=16 q-heads, K=8 kv-heads, H=256,
sliding window 1024, causal, soft-cap 50, query scale 0.0625, RoPE.

Sharding: 8 cores = 2 (batch) x 4 (head groups). Each core handles one
batch and 4 q-heads / 2 kv-heads (tensor parallel on the head axis of
q_w/kv_w/out_w). Host sums the 4 partial out-projections per batch.

On-device dataflow (all matmuls float32r = tf32-like at full PE rate):
  KV pass:  kT/vT = (x @ W)^T for both kv heads, RoPE on k.
  Q passes: qT per head in two 2-head passes (RoPE + scale fused).
  Attention (transposed, no running max needed thanks to the soft-cap
  bound): logits^T tiles [k,q], p = exp(50*tanh(L)-50), PV + column
  sums via a ones-matmul accumulate in PSUM, normalize with a
  partition-broadcast reciprocal -> encT.
  Out-proj: two 4-chunk passes; second accumulates into `out` via DMA.

Emission is software-pipelined: q-pass B interleaves with attention
group 0 (heads 0-1), out-proj pass A interleaves with attention group 1,
so TensorE keeps dense work while ScalarE runs the softmax chain.
"""

import os
import sys

sys.path.insert(0, "/opt/trn_rl_repo")

import numpy as np

B, T, D = 2, 2048, 3584
NQ, NKV, H = 16, 8, 256
P = 128
DC = D // P                 # 28 contraction chunks
HEADS_PER_CORE = 4
KV_PER_CORE = 2
SOFT_CAP = 50.0
SCALE = 0.0625
WINDOW = 1024
BASE_FREQ = 10000.0
QTILE = 512
NQT = T // QTILE            # 4
NKT = T // P                # 16

_NC_CACHE = {}
LAST_RESULTS = None


def _kt_list(qt):
    """Valid k-tiles for q-block qt with mask index (None = fully allowed)."""
    Q0 = qt * QTILE
    out = []
    for kt in range(NKT):
        K0 = kt * P
        if K0 > Q0 + QTILE - 1:
            continue
        if K0 + P - 1 <= Q0 - WINDOW:
            continue
        rel = K0 - Q0
        if rel >= 0:
            out.append((kt, rel // P))
        else:
            w = Q0 - K0 - WINDOW
            if -QTILE < w <= 0:
                out.append((kt, 4 + (-w) // P))
            else:
                out.append((kt, None))
    return out


def _make_masks():
    m = np.zeros((8, P, QTILE), np.float32)
    i = np.arange(P)[:, None]
    j = np.arange(QTILE)[None, :]
    for r in range(4):           # diag: allowed iff i <= j - rel
        m[r] = np.where(i <= j - r * P, 1.0, 0.0)
    for wi in range(4):          # window: allowed iff i > j + w
        m[4 + wi] = np.where(i > j - wi * P, 1.0, 0.0)
    return m


def _build_nc():
    import concourse.bacc as bacc
    import concourse.mybir as mybir
    import concourse.tile as tile
    from concourse.masks import make_identity

    f32 = mybir.dt.float32
    f32r = mybir.dt.float32r
    AF = mybir.ActivationFunctionType
    MULT = mybir.AluOpType.mult

    nc = bacc.Bacc()
    xT = nc.dram_tensor("xT", (D, T), f32r, kind="ExternalInput")
    qw = nc.dram_tensor("qw", (HEADS_PER_CORE, P, DC, H), f32r, kind="ExternalInput")
    kw = nc.dram_tensor("kw", (KV_PER_CORE, P, DC, H), f32r, kind="ExternalInput")
    vw = nc.dram_tensor("vw", (KV_PER_CORE, P, DC, H), f32r, kind="ExternalInput")
    ow = nc.dram_tensor("ow", (HEADS_PER_CORE, H, D), f32r, kind="ExternalInput")
    rope = nc.dram_tensor("rope", (P, 2, T), f32, kind="ExternalInput")
    msk = nc.dram_tensor("msk", (P, 8, QTILE), mybir.dt.bfloat16, kind="ExternalInput")
    out = nc.dram_tensor("out", (T, D), f32, kind="ExternalOutput")

    xTr = xT.rearrange("(c p) t -> p c t", p=P)
    QS = SCALE / SOFT_CAP

    with tile.TileContext(nc) as tc:
        dpool_cm = tc.tile_pool(name="dram", bufs=1, space="DRAM")
        dpool = dpool_cm.__enter__()
        qT = dpool.tile([HEADS_PER_CORE, P, 2, T], f32r)
        kT = dpool.tile([KV_PER_CORE, P, 2, T], f32r)
        vT = dpool.tile([KV_PER_CORE, P, 2, T], f32r)
        eT = dpool.tile([HEADS_PER_CORE, P, 2, T], f32r)

        # ---------------- P1: projections + RoPE ----------------
        xt_cm = tc.tile_pool(name="xt", bufs=4)
        xpool = xt_cm.__enter__()
        op_cm = tc.tile_pool(name="op", bufs=2)
        opool = op_cm.__enter__()
        w_cm = tc.tile_pool(name="w", bufs=1)
        wpool = w_cm.__enter__()
        rope_cm = tc.tile_pool(name="rope", bufs=1)
        rpool = rope_cm.__enter__()
        ps1_cm = tc.tile_pool(name="ps1", bufs=1, space="PSUM")
        ps1 = ps1_cm.__enter__()

        rope_sb = rpool.tile([P, 2, T], f32)
        nc.sync.dma_start(rope_sb[:], rope[:])
        cos_a = rope_sb[:, 0]
        sin_a = rope_sb[:, 1]

        wq0 = wpool.tile([P, DC, H], f32r, tag="wq0", name="wq0")
        nc.sync.dma_start(wq0[:], qw[0])

        for half in range(2):  # 0: k0,k1,v0,v1   1: q0..q3
            wts = []
            for j in range(4):
                if half == 1 and j == 0:
                    wts.append(wq0)
                    continue
                wt = wpool.tile([P, DC, H], f32r, tag=f"w{j}", name=f"w{j}")
                src = qw[j] if half == 1 else (kw[j] if j < 2 else vw[j - 2])
                nc.sync.dma_start(wt[:], src[:])
                wts.append(wt)
            for n in range(NQT):
                ns = slice(n * QTILE, (n + 1) * QTILE)
                psums = [
                    [
                        ps1.tile([P, QTILE], f32, tag=f"ps{j}{hc}",
                                 name=f"ps{j}{hc}")
                        for hc in range(2)
                    ]
                    for j in range(4)
                ]
                for dp in range(DC // 2):
                    # two D-chunks per DMA halves the sequencer issue load
                    xt = xpool.tile([P, 2, QTILE], f32r, tag="xt", name="xt")
                    nc.sync.dma_start(xt[:], xTr[:, 2 * dp : 2 * dp + 2, ns])
                    for u in range(2):
                        d = 2 * dp + u
                        for j in range(4):
                            for hc in range(2):
                                nc.tensor.matmul(
                                    psums[j][hc][:],
                                    wts[j][:, d, hc * P : (hc + 1) * P],
                                    xt[:, u],
                                    start=(d == 0),
                                    stop=(d == DC - 1),
                                )
                cos_t, sin_t = cos_a[:, ns], sin_a[:, ns]
                for j in range(4):
                    if half == 0 and j >= 2:  # v: copy out on idle ACT
                        for hc in range(2):
                            o = opool.tile([P, QTILE], f32r, tag=f"o{hc}",
                                           name="o")
                            nc.scalar.copy(o[:], psums[j][hc][:])
                            nc.sync.dma_start(vT[j - 2, :, hc, ns], o[:])
                        continue
                    c0 = opool.tile([P, QTILE], f32, tag="c0", name="c0")
                    s0 = opool.tile([P, QTILE], f32, tag="s0", name="s0")
                    c1 = opool.tile([P, QTILE], f32, tag="c1", name="c1")
                    s1 = opool.tile([P, QTILE], f32, tag="s1", name="s1")
                    o0 = opool.tile([P, QTILE], f32r, tag="o0", name="o0")
                    o1 = opool.tile([P, QTILE], f32r, tag="o1", name="o1")
                    p0, p1 = psums[j][0][:], psums[j][1][:]
                    if half == 1:  # q: fold SCALE/SOFT_CAP into the rotation
                        nc.vector.scalar_tensor_tensor(c0[:], p0, QS, cos_t, MULT, MULT)
                        nc.vector.scalar_tensor_tensor(s0[:], p0, QS, sin_t, MULT, MULT)
                        nc.vector.scalar_tensor_tensor(c1[:], p1, QS, cos_t, MULT, MULT)
                        nc.vector.scalar_tensor_tensor(s1[:], p1, QS, sin_t, MULT, MULT)
                    else:
                        nc.vector.tensor_mul(c0[:], p0, cos_t)
                        nc.vector.tensor_mul(s0[:], p0, sin_t)
                        nc.vector.tensor_mul(c1[:], p1, cos_t)
                        nc.vector.tensor_mul(s1[:], p1, sin_t)
                    nc.vector.tensor_sub(o0[:], c0[:], s1[:])
                    nc.vector.tensor_add(o1[:], c1[:], s0[:])
                    dstT = qT[j] if half == 1 else kT[j]
                    nc.sync.dma_start(dstT[:, 0, ns], o0[:])
                    nc.sync.dma_start(dstT[:, 1, ns], o1[:])

        ps1_cm.__exit__(None, None, None)
        rope_cm.__exit__(None, None, None)
        w_cm.__exit__(None, None, None)
        op_cm.__exit__(None, None, None)
        xt_cm.__exit__(None, None, None)

        # ---------------- P2: attention ----------------
        owp_cm = tc.tile_pool(name="owp", bufs=1)
        owp = owp_cm.__enter__()
        ow_sb = []
        for j in range(7):  # prefetch first 7 out-proj chunks during P2
            wt = owp.tile([P, D], f32r, tag=f"owp{j}", name=f"owp{j}")
            nc.sync.dma_start(wt[:], ow[j // 2, (j % 2) * P : (j % 2 + 1) * P, :])
            ow_sb.append(wt)

        cp_cm = tc.tile_pool(name="cp", bufs=1)
        cpool = cp_cm.__enter__()
        kv_cm = tc.tile_pool(name="kvp", bufs=1)
        kvpool = kv_cm.__enter__()
        qq_cm = tc.tile_pool(name="qq", bufs=3)
        qqpool = qq_cm.__enter__()
        sp_cm = tc.tile_pool(name="sp", bufs=2)
        spool = sp_cm.__enter__()
        ep_cm = tc.tile_pool(name="ep", bufs=2)
        epool = ep_cm.__enter__()
        psL_cm = tc.tile_pool(name="psL", bufs=2, space="PSUM")
        psL = psL_cm.__enter__()
        psA_cm = tc.tile_pool(name="psA", bufs=1, space="PSUM")
        psA = psA_cm.__enter__()

        masks_sb = cpool.tile([P, 8, QTILE], mybir.dt.bfloat16)
        nc.sync.dma_start(masks_sb[:], msk[:])
        ones_f = cpool.tile([P, 1], f32)
        nc.vector.memset(ones_f[:], 1.0)
        ones_r = cpool.tile([P, 1], f32r)
        nc.vector.tensor_copy(ones_r[:], ones_f[:])
        bias_m50 = cpool.tile([P, 1], f32)
        nc.vector.memset(bias_m50[:], -SOFT_CAP)
        idf = cpool.tile([P, P], f32)
        make_identity(nc, idf[:])
        idr = cpool.tile([P, P], f32r)
        nc.vector.tensor_copy(idr[:], idf[:])

        for kvh in range(KV_PER_CORE):
            kT_sb = kvpool.tile([P, 2, T], f32r, tag="kT", name="kT_sb")
            nc.sync.dma_start(kT_sb[:], kT[kvh])
            vT_sb = kvpool.tile([P, 2, T], f32r, tag="vT", name="vT_sb")
            nc.sync.dma_start(vT_sb[:], vT[kvh])
            v_all = kvpool.tile([P, NKT, H], f32r, tag="va", name="v_all")
            for kt in range(NKT):
                for hc in range(2):
                    pst = psL.tile([P, P], f32r, tag="L", name="pst")
                    nc.tensor.transpose(
                        pst[:], vT_sb[:, hc, kt * P : (kt + 1) * P], idr[:]
                    )
                    nc.vector.tensor_copy(
                        v_all[:, kt, hc * P : (hc + 1) * P], pst[:]
                    )
            for qh in (2 * kvh, 2 * kvh + 1):
                for qt in range(NQT):
                    qs = slice(qt * QTILE, (qt + 1) * QTILE)
                    qq = qqpool.tile([P, 2, QTILE], f32r, tag="qq", name="qq")
                    nc.sync.dma_start(qq[:], qT[qh][:, :, qs])
                    kts = _kt_list(qt)
                    db = qt % 2
                    enc_ps = [
                        psA.tile([P, QTILE], f32, tag=f"enc{hc}{db}",
                                 name="enc")
                        for hc in range(2)
                    ]
                    s_ps = psA.tile([1, QTILE], f32, tag=f"sums{db}",
                                    name="s_ps")
                    for i, (kt, mi) in enumerate(kts):
                        st, sp = (i == 0), (i == len(kts) - 1)
                        L = psL.tile([P, QTILE], f32, tag="L", name="L")
                        nc.tensor.matmul(
                            L[:], kT_sb[:, 0, kt * P : (kt + 1) * P], qq[:, 0],
                            start=True, stop=False,
                        )
                        nc.tensor.matmul(
                            L[:], kT_sb[:, 1, kt * P : (kt + 1) * P], qq[:, 1],
                            start=False, stop=True,
                        )
                        tt = spool.tile([P, QTILE], f32, tag="t", name="tt")
                        nc.scalar.activation(tt[:], L[:], AF.Tanh)
                        pp = spool.tile([P, QTILE], f32r, tag="p", name="pp")
                        nc.scalar.activation(
                            pp[:], tt[:], AF.Exp, bias=bias_m50[:],
                            scale=SOFT_CAP,
                        )
                        pu = pp[:]
                        if mi is not None:
                            pm = spool.tile([P, QTILE], f32r, tag="pm",
                                            name="pm")
                            nc.vector.tensor_mul(pm[:], pp[:], masks_sb[:, mi])
                            pu = pm[:]
                        nc.tensor.matmul(
                            enc_ps[0][:], v_all[:, kt, 0:P], pu,
                            start=st, stop=sp,
                        )
                        nc.tensor.matmul(
                            enc_ps[1][:], v_all[:, kt, P:H], pu,
                            start=st, stop=sp,
                        )
                        nc.tensor.matmul(
                            s_ps[:], ones_r[:], pu, start=st, stop=sp
                        )
                    rec = spool.tile([1, QTILE], f32, tag="rec", name="rec")
                    nc.vector.reciprocal(rec[:], s_ps[:])
                    rb = spool.tile([P, QTILE], f32, tag="rb", name="rb")
                    nc.gpsimd.partition_broadcast(rb[:], rec[:])
                    for hc in range(2):
                        eo = epool.tile([P, QTILE], f32r, tag=f"eo{hc}",
                                        name="eo")
                        nc.vector.tensor_mul(eo[:], enc_ps[hc][:], rb[:])
                        nc.sync.dma_start(eT[qh, :, hc, qs], eo[:])

        psA_cm.__exit__(None, None, None)
        psL_cm.__exit__(None, None, None)
        ep_cm.__exit__(None, None, None)
        sp_cm.__exit__(None, None, None)
        qq_cm.__exit__(None, None, None)
        kv_cm.__exit__(None, None, None)
        cp_cm.__exit__(None, None, None)

        # ---------------- P3: output projection ----------------
        ow2_cm = tc.tile_pool(name="ow2", bufs=1)
        ow2 = ow2_cm.__enter__()
        et_cm = tc.tile_pool(name="etp", bufs=2)
        etpool = et_cm.__enter__()
        o3_cm = tc.tile_pool(name="o3", bufs=2)
        o3pool = o3_cm.__enter__()
        po_cm = tc.tile_pool(name="po", bufs=3, space="PSUM")
        popool = po_cm.__enter__()

        for j in range(7, 8):
            wt = ow2.tile([P, D], f32r, tag=f"ow2{j}", name=f"ow2{j}")
            nc.sync.dma_start(
                wt[:], ow[j // 2, (j % 2) * P : (j % 2 + 1) * P, :]
            )
            ow_sb.append(wt)

        SPAN = 512
        for tci in range(T // P):
            ts_ = slice(tci * P, (tci + 1) * P)
            if tci % (SPAN // P) == 0:
                sp_ = slice(tci * P, tci * P + SPAN)
                ets = []
                for j in range(8):
                    et = etpool.tile([P, SPAN], f32r, tag=f"et{j}",
                                     name=f"et{j}")
                    nc.sync.dma_start(et[:], eT[j // 2, :, j % 2, sp_])
                    ets.append(et)
            off = (tci % (SPAN // P)) * P
            lhs = [e[:, off : off + P] for e in ets]
            out_sb = o3pool.tile([P, D], f32, tag="osb", name="osb")
            for nn in range(D // QTILE):
                nns = slice(nn * QTILE, (nn + 1) * QTILE)
                po = popool.tile([P, QTILE], f32, tag="po", name="po")
                for j in range(8):
                    nc.tensor.matmul(
                        po[:], lhs[j][:], ow_sb[j][:, nns],
                        start=(j == 0), stop=(j == 7),
                    )
                if nn % 2 == 0:
                    nc.vector.tensor_copy(out_sb[:, nns], po[:])
                else:
                    nc.scalar.copy(out_sb[:, nns], po[:])
            nc.sync.dma_start(out[ts_, :], out_sb[:])

        po_cm.__exit__(None, None, None)
        o3_cm.__exit__(None, None, None)
        et_cm.__exit__(None, None, None)
        ow2_cm.__exit__(None, None, None)
        owp_cm.__exit__(None, None, None)
        dpool_cm.__exit__(None, None, None)

    nc.finalize()
    return nc


def _install_axon_hooks_shim():
    """Provide antenv.axon_hooks if the image lacks it (NTFF profiling)."""
    import sys
    import types

    try:
        import antenv.axon_hooks  # noqa: F401

        return
    except ImportError:
        pass
    hook = None
    try:
        from trn_agent_boot.trn_boot import _ntff_profile_via_ctypes

        hook = _ntff_profile_via_ctypes("/opt/axon/libaxon_pjrt.so")
    except Exception:
        hook = None
    mod = types.ModuleType("antenv.axon_hooks")
    _h = [hook]
    mod.get_axon_ntff_profile_hook = lambda: _h[0]

    def _set(h):
        _h[0] = h

    mod.set_axon_ntff_profile_hook = _set
    sys.modules["antenv.axon_hooks"] = mod
    try:
        import antenv

        antenv.axon_hooks = mod
    except ImportError:
        pass


def _install_neff_cache():
    """Cache walrus-compiled NEFFs by BIR hash (compiles are minutes-long)."""
    import hashlib
    import shutil

    import concourse.bass2jax as b2j

    if getattr(b2j, "_ant_neff_cache_installed", False):
        return
    orig = b2j.compile_bir_kernel

    def cached(bir_json, tmpdir, neff_name="file.neff"):
        cdir = os.environ.get("NEFF_CACHE_DIR", "/tmp/neff_cache")
        os.makedirs(cdir, exist_ok=True)
        h = hashlib.sha256(bir_json).hexdigest()[:32]
        cpath = os.path.join(cdir, f"{h}.neff")
        if os.path.exists(cpath):
            dst = os.path.join(tmpdir, "sg00")
            os.makedirs(dst, exist_ok=True)
            dstf = os.path.join(dst, neff_name)
            shutil.copyfile(cpath, dstf)
            return dstf
        r = orig(bir_json, tmpdir, neff_name=neff_name)
        try:
            shutil.copyfile(r, cpath)
        except OSError:
            pass
        return r

    b2j.compile_bir_kernel = cached
    b2j._ant_neff_cache_installed = True


def kernel(x, segment_pos, attn_mask, q_w, kv_w, out_w):
    global LAST_RESULTS
    from concourse.bass_utils import run_bass_kernel_spmd

    _install_axon_hooks_shim()
    _install_neff_cache()

    x = np.asarray(x, np.float32)
    segment_pos = np.asarray(segment_pos, np.int32)
    q_w = np.asarray(q_w, np.float32)
    kv_w = np.asarray(kv_w, np.float32)
    out_w = np.asarray(out_w, np.float32)

    # RoPE tables per batch, host layout [P, 2, T]: [cos, sin]
    ropes = []
    for b in range(B):
        pos = segment_pos[b].astype(np.float32)
        fraction = 2.0 * np.arange(P, dtype=np.float32) / H
        timescale = BASE_FREQ**fraction
        ang = pos[None, :] / timescale[:, None]          # [128, T]
        r = np.stack([np.cos(ang), np.sin(ang)]).astype(np.float32)
        ropes.append(np.ascontiguousarray(r.transpose(1, 0, 2)))
    import ml_dtypes
    masks = np.ascontiguousarray(
        _make_masks().transpose(1, 0, 2).astype(ml_dtypes.bfloat16)
    )

    def _wlayout(w):
        # [nh, D, H] -> [nh, P, DC, H]: per-partition contiguous spans
        return np.ascontiguousarray(
            w.reshape(-1, DC, P, H).transpose(0, 2, 1, 3)
        )

    key = "main"
    if key not in _NC_CACHE:
        _NC_CACHE[key] = _build_nc()
    nc = _NC_CACHE[key]

    in_maps = []
    for core in range(8):
        b, g = core // 4, core % 4
        in_maps.append(
            {
                "xT": np.ascontiguousarray(x[b].T),
                "qw": _wlayout(q_w[4 * g : 4 * g + 4]),
                "kw": _wlayout(kv_w[0, 2 * g : 2 * g + 2]),
                "vw": _wlayout(kv_w[1, 2 * g : 2 * g + 2]),
                "ow": np.ascontiguousarray(out_w[4 * g : 4 * g + 4]),
                "rope": ropes[b],
                "msk": masks,
            }
        )

    res = run_bass_kernel_spmd(nc, in_maps, core_ids=list(range(8)))
    LAST_RESULTS = res

    outv = np.zeros((B, T, D), np.float32)
    for core in range(8):
        outv[core // 4] += res.results[core]["out"]
    return outv



# revision 4
# speedup vs baseline: 1.0233x; 1.0233x over previous
"""Trainium2 Bass kernel V2: GQA sliding-window attention, SBUF-resident bf16.

Problem: B=2, T=2048, D=3584, N=16 q-heads, K=8 kv-heads, H=256,
sliding window 1024, causal, soft-cap 50, query scale 0.0625, RoPE.

Sharding: 8 cores = 2 (batch) x 4 (head groups); each core: 4 q-heads,
2 kv-heads. Host sums the 4 partial out-projections per batch.

V2 design vs baseline:
  - All weights/intermediates bf16 (halves DMA + SBUF, full PE rate).
  - Zero DRAM round-trips: kT/qT/v/eT live in SBUF end-to-end.
  - V projected directly into [t, h] layout (stationary = x chunk), so
    no PE transposes and no copies.
  - SCALE/SOFT_CAP folded into q_w on host; RoPE drain = ACT copy
    (psum->bf16) + 6 bf16 DVE ops (4x mode) writing kT/qT in place.
  - Attention interleaves the two q-heads of each kv head to keep PE fed
    while the softmax chain (ACT tanh/exp, DVE mask-mul) runs.
  - Out-proj reads eT straight from SBUF, accumulating 8 matmuls in PSUM.
"""

import os
import sys

sys.path.insert(0, "/opt/trn_rl_repo")

import numpy as np

B, T, D = 2, 2048, 3584
NQ, NKV, H = 16, 8, 256
P = 128
DC = D // P                 # 28 contraction chunks
HEADS_PER_CORE = 4
KV_PER_CORE = 2
SOFT_CAP = 50.0
SCALE = 0.0625
WINDOW = 1024
BASE_FREQ = 10000.0
QTILE = 512
NQT = T // QTILE            # 4
ATILE = 256
NAT = T // ATILE            # 8
NKT = T // P                # 16

_NC_CACHE = {}
LAST_RESULTS = None


def _kt_list(at):
    """Valid k-tiles for q-block at (ATILE wide); mask index None = full."""
    Q0 = at * ATILE
    out = []
    for kt in range(NKT):
        K0 = kt * P
        if K0 > Q0 + ATILE - 1:
            continue
        if K0 + P - 1 <= Q0 - WINDOW:
            continue
        rel = K0 - Q0
        if rel >= 0:
            out.append((kt, rel // P))
        else:
            w = Q0 - K0 - WINDOW
            if -ATILE < w <= 0:
                out.append((kt, 2 + (-w) // P))
            else:
                out.append((kt, None))
    return out


def _make_masks():
    m = np.zeros((4, P, ATILE), np.float32)
    i = np.arange(P)[:, None]
    j = np.arange(ATILE)[None, :]
    for r in range(2):           # diag: allowed iff i <= j - rel
        m[r] = np.where(i <= j - r * P, 1.0, 0.0)
    for wi in range(2):          # window: allowed iff i > j - wi*128
        m[2 + wi] = np.where(i > j - wi * P, 1.0, 0.0)
    return m


def _build_nc():
    import concourse.bacc as bacc
    import concourse.mybir as mybir
    import concourse.tile as tile

    f32 = mybir.dt.float32
    bf16 = mybir.dt.bfloat16
    AF = mybir.ActivationFunctionType

    nc = bacc.Bacc()
    xT = nc.dram_tensor("xT", (P, DC, T), bf16, kind="ExternalInput")
    qw = nc.dram_tensor("qw", (HEADS_PER_CORE, P, DC, H), bf16,
                        kind="ExternalInput")
    kw = nc.dram_tensor("kw", (KV_PER_CORE, P, DC, H), bf16,
                        kind="ExternalInput")
    vw = nc.dram_tensor("vw", (P, DC, 2 * H), bf16, kind="ExternalInput")
    ow = nc.dram_tensor("ow", (2 * HEADS_PER_CORE, P, D), bf16,
                        kind="ExternalInput")
    rope = nc.dram_tensor("rope", (P, 2, T), bf16, kind="ExternalInput")
    msk = nc.dram_tensor("msk", (P, 4, ATILE), bf16, kind="ExternalInput")
    out = nc.dram_tensor("out", (T, D), f32, kind="ExternalOutput")

    with tile.TileContext(nc) as tc:
        pers_cm = tc.tile_pool(name="pers", bufs=1)
        pers = pers_cm.__enter__()

        # Persistent SBUF state
        kT = [pers.tile([P, 2, T], bf16, tag=f"kT{i}", name=f"kT{i}")
              for i in range(KV_PER_CORE)]
        qT = [pers.tile([P, 2, T], bf16, tag=f"qT{i}", name=f"qT{i}")
              for i in range(HEADS_PER_CORE)]
        vA = pers.tile([P, NKT, 2 * H], bf16, tag="vA", name="vA")
        masks_sb = pers.tile([P, 4, ATILE], bf16, tag="msk", name="masks_sb")
        ones_b = pers.tile([P, 1], bf16, tag="ones", name="ones_b")
        nc.vector.memset(ones_b[:], 1.0)
        bias_m50 = pers.tile([P, 1], f32, tag="b50", name="bias_m50")
        nc.vector.memset(bias_m50[:], -SOFT_CAP)
        # warm the GpSimd ucode library during P1 so the first real
        # partition_broadcast in attention doesn't stall ~10us on I$ load
        gwarm = pers.tile([P, 1], f32, tag="gw", name="gwarm")
        nc.gpsimd.partition_broadcast(gwarm[:], bias_m50[0:1, :])

        # ---------------- P1: projections + RoPE ----------------
        w_cm = tc.tile_pool(name="w", bufs=1)
        wpool = w_cm.__enter__()
        # rope table only lives through P1 — keep it in the P1 pool
        rope_sb = wpool.tile([P, 2, T], bf16, tag="rope", name="rope_sb")
        cos_a = rope_sb[:, 0]
        sin_a = rope_sb[:, 1]
        xt_cm = tc.tile_pool(name="xt", bufs=5)
        xpool = xt_cm.__enter__()
        tmp_cm = tc.tile_pool(name="tmp", bufs=2)
        tpool = tmp_cm.__enter__()
        ps1_cm = tc.tile_pool(name="ps1", bufs=1, space="PSUM")
        ps1 = ps1_cm.__enter__()

        for half in range(2):  # 0: k0,k1 + v(packed)   1: q0..q3
            # DMA issue order matters: every dma_start serializes on the
            # sync engine (~0.6us each), so issue the first-needed chunks
            # first: leading halves of the weights, then the first x tiles,
            # then the weight tails (and rope/masks, needed ~40us in).
            CSPLIT = 14

            if half == 0:
                # interleave weight-chunk and x-tile issue in contraction
                # order (geometric chunks): PE consumes ~380KB per d-group,
                # so arrival order must track the d loop or PE starves
                wk = [wpool.tile([P, DC, H], bf16, tag=f"w{j}",
                                 name=f"wk{j}") for j in range(2)]
                wv = wpool.tile([P, DC, 2 * H], bf16, tag="w2", name="wv")
                xt_pre = []

                def _xt_dma(d4):
                    xt = xpool.tile([P, 4, QTILE], bf16, tag="xt",
                                    name="xt")
                    nc.sync.dma_start(
                        xt[:], xT[:, 4 * d4 : 4 * d4 + 4, 0:QTILE]
                    )
                    xt_pre.append(xt)

                bounds = [0, 2, 6, 14, DC]
                for ci in range(4):
                    cs = slice(bounds[ci], bounds[ci + 1])
                    for j in range(2):
                        nc.sync.dma_start(wk[j][:, cs], kw[j][:, cs])
                    nc.sync.dma_start(wv[:, cs], vw[:, cs])
                    _xt_dma(ci)
                nc.sync.dma_start(rope_sb[:], rope[:])
                nc.sync.dma_start(masks_sb[:], msk[:])
            else:
                wq = []
                for j in range(4):
                    wt = wpool.tile([P, DC, H], bf16, tag=f"w{j}",
                                    name=f"wq{j}")
                    nc.sync.dma_start(wt[:], qw[j])
                    wq.append(wt)

            for n in range(NQT):
                ns = slice(n * QTILE, (n + 1) * QTILE)
                if half == 0:
                    # k psums: tags t0..t3; v psums (per t-chunk): t4..t7
                    kps = [
                        [ps1.tile([P, QTILE], f32, tag=f"t{2 * j + hc}",
                                  name=f"kps{j}{hc}") for hc in range(2)]
                        for j in range(2)
                    ]
                    vps = [ps1.tile([P, 2 * H], f32, tag=f"t{4 + tci}",
                                    name=f"vps{tci}") for tci in range(4)]
                else:
                    qps = [
                        [ps1.tile([P, QTILE], f32, tag=f"t{2 * j + hc}",
                                  name=f"qps{j}{hc}") for hc in range(2)]
                        for j in range(4)
                    ]
                for dp in range(DC // 4):
                    if half == 0 and n == 0 and dp < 4:
                        xt = xt_pre[dp]
                    else:
                        xt = xpool.tile([P, 4, QTILE], bf16, tag="xt",
                                        name="xt")
                        nc.sync.dma_start(
                            xt[:], xT[:, 4 * dp : 4 * dp + 4, ns]
                        )
                    for u in range(4):
                        d = 4 * dp + u
                        st, sp = (d == 0), (d == DC - 1)
                        if half == 0:
                            for j in range(2):
                                for hc in range(2):
                                    nc.tensor.matmul(
                                        kps[j][hc][:],
                                        wk[j][:, d, hc * P : (hc + 1) * P],
                                        xt[:, u], start=st, stop=sp,
                                    )
                            for tci in range(4):
                                nc.tensor.matmul(
                                    vps[tci][:],
                                    xt[:, u, tci * P : (tci + 1) * P],
                                    wv[:, d, :], start=st, stop=sp,
                                )
                        else:
                            for j in range(4):
                                for hc in range(2):
                                    nc.tensor.matmul(
                                        qps[j][hc][:],
                                        wq[j][:, d, hc * P : (hc + 1) * P],
                                        xt[:, u], start=st, stop=sp,
                                    )
                # drains
                cos_t, sin_t = cos_a[:, ns], sin_a[:, ns]
                if half == 0:
                    for tci in range(4):
                        tg = n * 4 + tci
                        nc.scalar.copy(vA[:, tg, :], vps[tci][:])
                    rope_sets = [(j, kps[j], kT[j]) for j in range(2)]
                else:
                    rope_sets = [(j, qps[j], qT[j]) for j in range(4)]
                for j, ps, dstT in rope_sets:
                    t0 = tpool.tile([P, QTILE], bf16, tag="t0", name="t0")
                    t1 = tpool.tile([P, QTILE], bf16, tag="t1", name="t1")
                    nc.scalar.copy(t0[:], ps[0][:])
                    nc.vector.tensor_copy(t1[:], ps[1][:])
                    c0 = tpool.tile([P, QTILE], bf16, tag="c0", name="c0")
                    s0 = tpool.tile([P, QTILE], bf16, tag="s0", name="s0")
                    c1 = tpool.tile([P, QTILE], bf16, tag="c1", name="c1")
                    s1 = tpool.tile([P, QTILE], bf16, tag="s1", name="s1")
                    nc.vector.tensor_mul(c0[:], t0[:], cos_t)
                    nc.vector.tensor_mul(s0[:], t0[:], sin_t)
                    nc.vector.tensor_mul(c1[:], t1[:], cos_t)
                    nc.vector.tensor_mul(s1[:], t1[:], sin_t)
                    nc.vector.tensor_sub(dstT[:, 0, ns], c0[:], s1[:])
                    nc.vector.tensor_add(dstT[:, 1, ns], c1[:], s0[:])

        ps1_cm.__exit__(None, None, None)
        tmp_cm.__exit__(None, None, None)
        xt_cm.__exit__(None, None, None)
        w_cm.__exit__(None, None, None)

        # ---------------- P2: attention ----------------
        owp_cm = tc.tile_pool(name="owp", bufs=1)
        owp = owp_cm.__enter__()
        ow_sb = []
        for j in range(2 * HEADS_PER_CORE):
            wt = owp.tile([P, D], bf16, tag=f"owp{j}", name=f"owp{j}")
            nc.sync.dma_start(wt[:], ow[j])
            ow_sb.append(wt)

        et_cm = tc.tile_pool(name="et", bufs=1)
        etp = et_cm.__enter__()
        eT = [etp.tile([P, 2, T], bf16, tag=f"eT{i}", name=f"eT{i}")
              for i in range(HEADS_PER_CORE)]

        sp_cm = tc.tile_pool(name="sp", bufs=3)
        spool = sp_cm.__enter__()
        np_cm = tc.tile_pool(name="np", bufs=2)
        npool = np_cm.__enter__()
        psL_cm = tc.tile_pool(name="psL", bufs=2, space="PSUM")
        psL = psL_cm.__enter__()
        psE_cm = tc.tile_pool(name="psE", bufs=1, space="PSUM")
        psE = psE_cm.__enter__()
        o3_cm = tc.tile_pool(name="o3", bufs=3)
        o3pool = o3_cm.__enter__()
        po_cm = tc.tile_pool(name="po", bufs=3, space="PSUM")
        popool = po_cm.__enter__()

        # attention and out-projection are emitted q-block-major and share
        # the PSUM banks (2 L + 2 enc + 1 sums + 3 out-proj = 8): out-proj
        # matmuls of q-block at are ready work that fills PE bubbles while
        # q-block at+1's softmax chains (ACT tanh/exp) drain.
        for at in range(NAT):
            qs = slice(at * ATILE, (at + 1) * ATILE)
            kts = _kt_list(at)
            for qh in range(HEADS_PER_CORE):
                kvh = qh // 2
                kTh = kT[kvh]
                enc_ps = [
                    psE.tile([P, ATILE], f32, tag=f"enc{hc}",
                             name=f"enc{hc}")
                    for hc in range(2)
                ]
                s_ps = psE.tile([1, ATILE], f32, tag="s", name="s")
                for i, (kt, mi) in enumerate(kts):
                    st, sp = (i == 0), (i == len(kts) - 1)
                    L = psL.tile([P, ATILE], f32, tag="L", name="L")
                    nc.tensor.matmul(
                        L[:], kTh[:, 0, kt * P : (kt + 1) * P],
                        qT[qh][:, 0, qs], start=True, stop=False,
                    )
                    nc.tensor.matmul(
                        L[:], kTh[:, 1, kt * P : (kt + 1) * P],
                        qT[qh][:, 1, qs], start=False, stop=True,
                    )
                    tt = spool.tile([P, ATILE], f32, tag="tt", name="tt")
                    nc.scalar.activation(tt[:], L[:], AF.Tanh)
                    pp = spool.tile([P, ATILE], bf16, tag="pp", name="pp")
                    nc.scalar.activation(
                        pp[:], tt[:], AF.Exp, bias=bias_m50[:],
                        scale=SOFT_CAP,
                    )
                    pu = pp[:]
                    if mi is not None:
                        pm = spool.tile([P, ATILE], bf16, tag="pm",
                                        name="pm")
                        nc.vector.tensor_mul(pm[:], pp[:], masks_sb[:, mi])
                        pu = pm[:]
                    base = kvh * H
                    nc.tensor.matmul(
                        enc_ps[0][:], vA[:, kt, base : base + P], pu,
                        start=st, stop=sp,
                    )
                    nc.tensor.matmul(
                        enc_ps[1][:], vA[:, kt, base + P : base + 2 * P],
                        pu, start=st, stop=sp,
                    )
                    nc.tensor.matmul(
                        s_ps[:], ones_b[:], pu, start=st, stop=sp,
                    )
                # evacuate enc PSUM banks promptly (DVE copies) so the
                # next head's PV accumulation isn't gated on the
                # reciprocal/broadcast chain
                ecs = []
                for hc in range(2):
                    ec = npool.tile([P, ATILE], f32, tag=f"ec{hc}",
                                    name="ec")
                    nc.vector.tensor_copy(ec[:], enc_ps[hc][:])
                    ecs.append(ec)
                rec = npool.tile([1, ATILE], f32, tag="rec", name="rec")
                nc.vector.reciprocal_approx_fast(rec[:], s_ps[:])
                rb = npool.tile([P, ATILE], f32, tag="rb", name="rb")
                nc.gpsimd.partition_broadcast(rb[:], rec[:])
                for hc in range(2):
                    nc.vector.tensor_mul(
                        eT[qh][:, hc, qs], ecs[hc][:], rb[:]
                    )
            # out-projection for this q-block's two t-chunks
            for tci in range(2 * at, 2 * at + 2):
                ts_ = slice(tci * P, (tci + 1) * P)
                for nn in range(D // QTILE):
                    nns = slice(nn * QTILE, (nn + 1) * QTILE)
                    po = popool.tile([P, QTILE], f32, tag="po", name="po")
                    for j in range(2 * HEADS_PER_CORE):
                        nc.tensor.matmul(
                            po[:], eT[j // 2][:, j % 2, ts_],
                            ow_sb[j][:, nns],
                            start=(j == 0),
                            stop=(j == 2 * HEADS_PER_CORE - 1),
                        )
                    ob = o3pool.tile([P, QTILE], f32, tag="osb", name="osb")
                    nc.scalar.copy(ob[:], po[:])
                    nc.sync.dma_start(out[ts_, nns], ob[:])

        po_cm.__exit__(None, None, None)
        o3_cm.__exit__(None, None, None)
        psE_cm.__exit__(None, None, None)
        psL_cm.__exit__(None, None, None)
        np_cm.__exit__(None, None, None)
        sp_cm.__exit__(None, None, None)
        et_cm.__exit__(None, None, None)
        owp_cm.__exit__(None, None, None)
        pers_cm.__exit__(None, None, None)

    nc.finalize()
    return nc


def _install_axon_hooks_shim():
    """Provide antenv.axon_hooks if the image lacks it (NTFF profiling)."""
    import types

    try:
        import antenv.axon_hooks  # noqa: F401

        return
    except ImportError:
        pass
    hook = None
    try:
        from trn_agent_boot.trn_boot import _ntff_profile_via_ctypes

        hook = _ntff_profile_via_ctypes("/opt/axon/libaxon_pjrt.so")
    except Exception:
        hook = None
    mod = types.ModuleType("antenv.axon_hooks")
    _h = [hook]
    mod.get_axon_ntff_profile_hook = lambda: _h[0]

    def _set(h):
        _h[0] = h

    mod.set_axon_ntff_profile_hook = _set
    sys.modules["antenv.axon_hooks"] = mod
    try:
        import antenv

        antenv.axon_hooks = mod
    except ImportError:
        pass


def _install_neff_cache():
    """Cache walrus-compiled NEFFs by BIR hash (compiles are minutes-long)."""
    import hashlib
    import shutil

    import concourse.bass2jax as b2j

    if getattr(b2j, "_ant_neff_cache_installed", False):
        return
    orig = b2j.compile_bir_kernel

    def cached(bir_json, tmpdir, neff_name="file.neff"):
        cdir = os.environ.get("NEFF_CACHE_DIR", "/tmp/neff_cache")
        os.makedirs(cdir, exist_ok=True)
        h = hashlib.sha256(bir_json).hexdigest()[:32]
        cpath = os.path.join(cdir, f"{h}.neff")
        if os.path.exists(cpath):
            dst = os.path.join(tmpdir, "sg00")
            os.makedirs(dst, exist_ok=True)
            dstf = os.path.join(dst, neff_name)
            shutil.copyfile(cpath, dstf)
            return dstf
        r = orig(bir_json, tmpdir, neff_name=neff_name)
        try:
            shutil.copyfile(r, cpath)
        except OSError:
            pass
        return r

    b2j.compile_bir_kernel = cached
    b2j._ant_neff_cache_installed = True


def _host_inputs(x, segment_pos, q_w, kv_w, out_w):
    """Per-core input maps (bf16 host-side prep)."""
    import ml_dtypes

    bf = ml_dtypes.bfloat16
    QS = SCALE / SOFT_CAP

    def _wlayout(w):
        # [nh, D, H] -> [nh, P, DC, H]: per-partition contiguous spans
        return np.ascontiguousarray(
            w.reshape(-1, DC, P, w.shape[-1]).transpose(0, 2, 1, 3)
        ).astype(bf)

    ropes = []
    for b in range(B):
        pos = segment_pos[b].astype(np.float32)
        fraction = 2.0 * np.arange(P, dtype=np.float32) / H
        timescale = BASE_FREQ**fraction
        ang = pos[None, :] / timescale[:, None]          # [128, T]
        r = np.stack([np.cos(ang), np.sin(ang)])
        ropes.append(
            np.ascontiguousarray(r.transpose(1, 0, 2)).astype(bf)
        )
    masks = np.ascontiguousarray(
        _make_masks().transpose(1, 0, 2)
    ).astype(bf)

    xTs = []
    for b in range(B):
        xt = np.ascontiguousarray(
            x[b].T.reshape(DC, P, T).transpose(1, 0, 2)
        ).astype(bf)
        xTs.append(xt)

    in_maps = []
    for core in range(8):
        b, g = core // 4, core % 4
        qws = _wlayout(q_w[4 * g : 4 * g + 4] * QS)
        kws = _wlayout(kv_w[0, 2 * g : 2 * g + 2])
        # pack both v heads along H: [P, DC, 2H]
        vss = _wlayout(kv_w[1, 2 * g : 2 * g + 2])   # [2, P, DC, H]
        vwp = np.ascontiguousarray(
            np.concatenate([vss[0], vss[1]], axis=-1)
        )
        ows = np.ascontiguousarray(
            out_w[4 * g : 4 * g + 4].reshape(2 * HEADS_PER_CORE, P, D)
        ).astype(bf)
        in_maps.append(
            {
                "xT": xTs[b],
                "qw": qws,
                "kw": kws,
                "vw": vwp,
                "ow": ows,
                "rope": ropes[b],
                "msk": masks,
            }
        )
    return in_maps


def kernel(x, segment_pos, attn_mask, q_w, kv_w, out_w):
    global LAST_RESULTS
    from concourse.bass_utils import run_bass_kernel_spmd

    _install_axon_hooks_shim()
    _install_neff_cache()

    x = np.asarray(x, np.float32)
    segment_pos = np.asarray(segment_pos, np.int32)
    q_w = np.asarray(q_w, np.float32)
    kv_w = np.asarray(kv_w, np.float32)
    out_w = np.asarray(out_w, np.float32)

    key = "main"
    if key not in _NC_CACHE:
        _NC_CACHE[key] = _build_nc()
    nc = _NC_CACHE[key]

    in_maps = _host_inputs(x, segment_pos, q_w, kv_w, out_w)
    res = run_bass_kernel_spmd(nc, in_maps, core_ids=list(range(8)))
    LAST_RESULTS = res

    outv = np.zeros((B, T, D), np.float32)
    for core in range(8):
        outv[core // 4] += res.results[core]["out"]
    return outv


# revision 5
# speedup vs baseline: 1.0709x; 1.0465x over previous
"""Trainium2 Bass kernel V2: GQA sliding-window attention, SBUF-resident bf16.

Problem: B=2, T=2048, D=3584, N=16 q-heads, K=8 kv-heads, H=256,
sliding window 1024, causal, soft-cap 50, query scale 0.0625, RoPE.

Sharding: 8 cores = 2 (batch) x 4 (head groups); each core: 4 q-heads,
2 kv-heads. Host sums the 4 partial out-projections per batch.

V2 design vs baseline:
  - All weights/intermediates bf16 (halves DMA + SBUF, full PE rate).
  - Zero DRAM round-trips: kT/qT/v/eT live in SBUF end-to-end.
  - V projected directly into [t, h] layout (stationary = x chunk), so
    no PE transposes and no copies.
  - SCALE/SOFT_CAP folded into q_w on host; RoPE drain = ACT copy
    (psum->bf16) + 6 bf16 DVE ops (4x mode) writing kT/qT in place.
  - Attention interleaves the two q-heads of each kv head to keep PE fed
    while the softmax chain (ACT tanh/exp, DVE mask-mul) runs.
  - Out-proj reads eT straight from SBUF, accumulating 8 matmuls in PSUM.
"""

import os
import sys

sys.path.insert(0, "/opt/trn_rl_repo")

import numpy as np

B, T, D = 2, 2048, 3584
NQ, NKV, H = 16, 8, 256
P = 128
DC = D // P                 # 28 contraction chunks
HEADS_PER_CORE = 4
KV_PER_CORE = 2
SOFT_CAP = 50.0
SCALE = 0.0625
WINDOW = 1024
BASE_FREQ = 10000.0
QTILE = 512
NQT = T // QTILE            # 4
ATILE = 256
NAT = T // ATILE            # 8
NKT = T // P                # 16

_NC_CACHE = {}
LAST_RESULTS = None


def _kt_list(at):
    """Valid k-tiles for q-block at (ATILE wide); mask index None = full."""
    Q0 = at * ATILE
    out = []
    for kt in range(NKT):
        K0 = kt * P
        if K0 > Q0 + ATILE - 1:
            continue
        if K0 + P - 1 <= Q0 - WINDOW:
            continue
        rel = K0 - Q0
        if rel >= 0:
            out.append((kt, rel // P))
        else:
            w = Q0 - K0 - WINDOW
            if -ATILE < w <= 0:
                out.append((kt, 2 + (-w) // P))
            else:
                out.append((kt, None))
    return out


def _make_masks():
    m = np.zeros((4, P, ATILE), np.float32)
    i = np.arange(P)[:, None]
    j = np.arange(ATILE)[None, :]
    for r in range(2):           # diag: allowed iff i <= j - rel
        m[r] = np.where(i <= j - r * P, 1.0, 0.0)
    for wi in range(2):          # window: allowed iff i > j - wi*128
        m[2 + wi] = np.where(i > j - wi * P, 1.0, 0.0)
    return m


def _build_nc():
    import concourse.bacc as bacc
    import concourse.mybir as mybir
    import concourse.tile as tile
    from concourse import bass_isa

    f32 = mybir.dt.float32
    bf16 = mybir.dt.bfloat16
    AF = mybir.ActivationFunctionType

    nc = bacc.Bacc()
    xT = nc.dram_tensor("xT", (P, DC, T), bf16, kind="ExternalInput")
    qw = nc.dram_tensor("qw", (HEADS_PER_CORE, P, DC, H), bf16,
                        kind="ExternalInput")
    kw = nc.dram_tensor("kw", (KV_PER_CORE, P, DC, H), bf16,
                        kind="ExternalInput")
    vw = nc.dram_tensor("vw", (P, DC, 2 * H), bf16, kind="ExternalInput")
    ow = nc.dram_tensor("ow", (2 * HEADS_PER_CORE, P, D), bf16,
                        kind="ExternalInput")
    rope = nc.dram_tensor("rope", (P, 2, T), bf16, kind="ExternalInput")
    msk = nc.dram_tensor("msk", (P, 4, 2 * ATILE), bf16,
                         kind="ExternalInput")
    out = nc.dram_tensor("out", (T, D), f32, kind="ExternalOutput")

    with tile.TileContext(nc) as tc:
        pers_cm = tc.tile_pool(name="pers", bufs=1)
        pers = pers_cm.__enter__()

        # Persistent SBUF state
        kT = [pers.tile([P, 2, T], bf16, tag=f"kT{i}", name=f"kT{i}")
              for i in range(KV_PER_CORE)]
        # q stored pair-packed: [hc, at-block, head-in-pair, ATILE] so one
        # 512-wide moving operand covers both heads of a kv pair
        qP = [pers.tile([P, 2, NAT, 2, ATILE], bf16, tag=f"qP{i}",
                        name=f"qP{i}") for i in range(KV_PER_CORE)]
        vA = pers.tile([P, NKT, 2 * H], bf16, tag="vA", name="vA")
        masks_sb = pers.tile([P, 4, 2 * ATILE], bf16, tag="msk",
                             name="masks_sb")
        bias_m50 = pers.tile([P, 1], f32, tag="b50", name="bias_m50")
        nc.vector.memset(bias_m50[:], -SOFT_CAP)
        # warm the GpSimd ucode library during P1 so the first real
        # partition_all_reduce in attention doesn't stall ~10us on I$ load
        gwarm = pers.tile([P, 1], f32, tag="gw", name="gwarm")
        nc.gpsimd.partition_all_reduce(gwarm[:], bias_m50[:], P,
                                       bass_isa.ReduceOp.add)

        # ---------------- P1: projections + RoPE ----------------
        w_cm = tc.tile_pool(name="w", bufs=1)
        wpool = w_cm.__enter__()
        # rope table only lives through P1 — keep it in the P1 pool
        rope_sb = wpool.tile([P, 2, T], bf16, tag="rope", name="rope_sb")
        cos_a = rope_sb[:, 0]
        sin_a = rope_sb[:, 1]
        xt_cm = tc.tile_pool(name="xt", bufs=5)
        xpool = xt_cm.__enter__()
        tmp_cm = tc.tile_pool(name="tmp", bufs=2)
        tpool = tmp_cm.__enter__()
        ps1_cm = tc.tile_pool(name="ps1", bufs=1, space="PSUM")
        ps1 = ps1_cm.__enter__()

        for half in range(2):  # 0: k0,k1 + v(packed)   1: q0..q3
            # DMA issue order matters: every dma_start serializes on the
            # sync engine (~0.6us each), so issue the first-needed chunks
            # first: leading halves of the weights, then the first x tiles,
            # then the weight tails (and rope/masks, needed ~40us in).
            CSPLIT = 14

            if half == 0:
                # interleave weight-chunk and x-tile issue in contraction
                # order (geometric chunks): PE consumes ~380KB per d-group,
                # so arrival order must track the d loop or PE starves
                wk = [wpool.tile([P, DC, H], bf16, tag=f"w{j}",
                                 name=f"wk{j}") for j in range(2)]
                wv = wpool.tile([P, DC, 2 * H], bf16, tag="w2", name="wv")
                xt_pre = []

                def _xt_dma(d4):
                    xt = xpool.tile([P, 4, QTILE], bf16, tag="xt",
                                    name="xt")
                    nc.sync.dma_start(
                        xt[:], xT[:, 4 * d4 : 4 * d4 + 4, 0:QTILE]
                    )
                    xt_pre.append(xt)

                bounds = [0, 2, 6, 14, DC]
                for ci in range(4):
                    cs = slice(bounds[ci], bounds[ci + 1])
                    for j in range(2):
                        nc.sync.dma_start(wk[j][:, cs], kw[j][:, cs])
                    nc.sync.dma_start(wv[:, cs], vw[:, cs])
                    _xt_dma(ci)
                nc.sync.dma_start(rope_sb[:], rope[:])
                nc.sync.dma_start(masks_sb[:], msk[:])
            else:
                wq = []
                for j in range(4):
                    wt = wpool.tile([P, DC, H], bf16, tag=f"w{j}",
                                    name=f"wq{j}")
                    nc.sync.dma_start(wt[:], qw[j])
                    wq.append(wt)

            for n in range(NQT):
                ns = slice(n * QTILE, (n + 1) * QTILE)
                if half == 0:
                    # k psums: tags t0..t3; v psums (per t-chunk): t4..t7
                    kps = [
                        [ps1.tile([P, QTILE], f32, tag=f"t{2 * j + hc}",
                                  name=f"kps{j}{hc}") for hc in range(2)]
                        for j in range(2)
                    ]
                    vps = [ps1.tile([P, 2 * H], f32, tag=f"t{4 + tci}",
                                    name=f"vps{tci}") for tci in range(4)]
                else:
                    qps = [
                        [ps1.tile([P, QTILE], f32, tag=f"t{2 * j + hc}",
                                  name=f"qps{j}{hc}") for hc in range(2)]
                        for j in range(4)
                    ]
                for dp in range(DC // 4):
                    if half == 0 and n == 0 and dp < 4:
                        xt = xt_pre[dp]
                    else:
                        xt = xpool.tile([P, 4, QTILE], bf16, tag="xt",
                                        name="xt")
                        nc.sync.dma_start(
                            xt[:], xT[:, 4 * dp : 4 * dp + 4, ns]
                        )
                    for u in range(4):
                        d = 4 * dp + u
                        st, sp = (d == 0), (d == DC - 1)
                        if half == 0:
                            for j in range(2):
                                for hc in range(2):
                                    nc.tensor.matmul(
                                        kps[j][hc][:],
                                        wk[j][:, d, hc * P : (hc + 1) * P],
                                        xt[:, u], start=st, stop=sp,
                                    )
                            for tci in range(4):
                                nc.tensor.matmul(
                                    vps[tci][:],
                                    xt[:, u, tci * P : (tci + 1) * P],
                                    wv[:, d, :], start=st, stop=sp,
                                )
                        else:
                            for j in range(4):
                                for hc in range(2):
                                    nc.tensor.matmul(
                                        qps[j][hc][:],
                                        wq[j][:, d, hc * P : (hc + 1) * P],
                                        xt[:, u], start=st, stop=sp,
                                    )
                # drains
                cos_t, sin_t = cos_a[:, ns], sin_a[:, ns]
                if half == 0:
                    for tci in range(4):
                        tg = n * 4 + tci
                        nc.scalar.copy(vA[:, tg, :], vps[tci][:])
                    rope_sets = [(j, kps[j]) for j in range(2)]
                else:
                    rope_sets = [(j, qps[j]) for j in range(4)]
                for j, ps in rope_sets:
                    t0 = tpool.tile([P, QTILE], bf16, tag="t0", name="t0")
                    t1 = tpool.tile([P, QTILE], bf16, tag="t1", name="t1")
                    nc.scalar.copy(t0[:], ps[0][:])
                    nc.vector.tensor_copy(t1[:], ps[1][:])
                    c0 = tpool.tile([P, QTILE], bf16, tag="c0", name="c0")
                    s0 = tpool.tile([P, QTILE], bf16, tag="s0", name="s0")
                    c1 = tpool.tile([P, QTILE], bf16, tag="c1", name="c1")
                    s1 = tpool.tile([P, QTILE], bf16, tag="s1", name="s1")
                    nc.vector.tensor_mul(c0[:], t0[:], cos_t)
                    nc.vector.tensor_mul(s0[:], t0[:], sin_t)
                    nc.vector.tensor_mul(c1[:], t1[:], cos_t)
                    nc.vector.tensor_mul(s1[:], t1[:], sin_t)
                    if half == 0:
                        nc.vector.tensor_sub(kT[j][:, 0, ns], c0[:], s1[:])
                        nc.vector.tensor_add(kT[j][:, 1, ns], c1[:], s0[:])
                    else:
                        pr, ab = j // 2, j % 2
                        for h2 in range(2):
                            cs = slice(h2 * ATILE, (h2 + 1) * ATILE)
                            nc.vector.tensor_sub(
                                qP[pr][:, 0, 2 * n + h2, ab],
                                c0[:, cs], s1[:, cs],
                            )
                            nc.vector.tensor_add(
                                qP[pr][:, 1, 2 * n + h2, ab],
                                c1[:, cs], s0[:, cs],
                            )

        ps1_cm.__exit__(None, None, None)
        tmp_cm.__exit__(None, None, None)
        xt_cm.__exit__(None, None, None)
        w_cm.__exit__(None, None, None)

        # ---------------- P2: attention ----------------
        owp_cm = tc.tile_pool(name="owp", bufs=1)
        owp = owp_cm.__enter__()
        ow_sb = []
        for j in range(2 * HEADS_PER_CORE):
            wt = owp.tile([P, D], bf16, tag=f"owp{j}", name=f"owp{j}")
            nc.sync.dma_start(wt[:], ow[j])
            ow_sb.append(wt)

        et_cm = tc.tile_pool(name="et", bufs=1)
        etp = et_cm.__enter__()
        eT = [etp.tile([P, 2, T], bf16, tag=f"eT{i}", name=f"eT{i}")
              for i in range(HEADS_PER_CORE)]

        sp_cm = tc.tile_pool(name="sp", bufs=3)
        spool = sp_cm.__enter__()
        np_cm = tc.tile_pool(name="np", bufs=2)
        npool = np_cm.__enter__()
        psL_cm = tc.tile_pool(name="psL", bufs=3, space="PSUM")
        psL = psL_cm.__enter__()
        psE_cm = tc.tile_pool(name="psE", bufs=1, space="PSUM")
        psE = psE_cm.__enter__()
        o3_cm = tc.tile_pool(name="o3", bufs=3)
        o3pool = o3_cm.__enter__()
        po_cm = tc.tile_pool(name="po", bufs=3, space="PSUM")
        popool = po_cm.__enter__()

        # attention and out-projection are emitted q-block-major and share
        # PSUM (3 L + 2 enc + 3 out-proj = 8 banks). The two q-heads of
        # each kv head are CONCATENATED along the free dim (qA|qB, 512
        # wide), so QK, tanh, exp, mask-mul, PV, and the denominator chain
        # each run once per k-tile for both heads at full 512-wide rates.
        # Denominator: DVE accumulate + GpSimd partition_all_reduce (no
        # ones-matmul, no PSUM bank, no 1-partition reciprocal).
        W2 = 2 * ATILE
        for at in range(NAT):
            kts = _kt_list(at)
            for pair in range(2):
                kvh = pair
                kTh = kT[kvh]
                enc_ps = [
                    psE.tile([P, W2], f32, tag=f"enc{hc}", name=f"enc{hc}")
                    for hc in range(2)
                ]
                acc = npool.tile([P, W2], f32, tag="acc", name="acc")
                for i, (kt, mi) in enumerate(kts):
                    st, sp = (i == 0), (i == len(kts) - 1)
                    L = psL.tile([P, W2], f32, tag="L", name="L")
                    for hc in range(2):
                        nc.tensor.matmul(
                            L[:], kTh[:, hc, kt * P : (kt + 1) * P],
                            qP[pair][:, hc, at], start=(hc == 0),
                            stop=(hc == 1),
                        )
                    tt = spool.tile([P, W2], f32, tag="tt", name="tt")
                    nc.scalar.activation(tt[:], L[:], AF.Tanh)
                    pp = spool.tile([P, W2], bf16, tag="pp", name="pp")
                    nc.scalar.activation(
                        pp[:], tt[:], AF.Exp, bias=bias_m50[:],
                        scale=SOFT_CAP,
                    )
                    pu = pp[:]
                    if mi is not None:
                        pm = spool.tile([P, W2], bf16, tag="pm", name="pm")
                        nc.vector.tensor_mul(pm[:], pp[:], masks_sb[:, mi])
                        pu = pm[:]
                    if st:
                        nc.vector.tensor_copy(acc[:], pu)
                    else:
                        nc.vector.tensor_add(acc[:], acc[:], pu)
                    base = kvh * H
                    for hc in range(2):
                        nc.tensor.matmul(
                            enc_ps[hc][:],
                            vA[:, kt, base + hc * P : base + (hc + 1) * P],
                            pu, start=st, stop=sp,
                        )
                # evacuate enc PSUM banks promptly (DVE copies), then
                # normalize both heads at once
                ecs = []
                for hc in range(2):
                    ec = npool.tile([P, W2], f32, tag=f"ec{hc}", name="ec")
                    nc.vector.tensor_copy(ec[:], enc_ps[hc][:])
                    ecs.append(ec)
                dent = npool.tile([P, W2], f32, tag="dn", name="dent")
                nc.gpsimd.partition_all_reduce(
                    dent[:], acc[:], P, bass_isa.ReduceOp.add
                )
                rb = npool.tile([P, W2], f32, tag="rb", name="rb")
                nc.vector.reciprocal_approx_fast(rb[:], dent[:])
                for ab in range(2):
                    h = 2 * pair + ab
                    qs = slice(at * ATILE, (at + 1) * ATILE)
                    cs = slice(ab * ATILE, (ab + 1) * ATILE)
                    for hc in range(2):
                        nc.vector.tensor_mul(
                            eT[h][:, hc, qs], ecs[hc][:, cs], rb[:, cs]
                        )
            # out-projection for this q-block's two t-chunks
            for tci in range(2 * at, 2 * at + 2):
                ts_ = slice(tci * P, (tci + 1) * P)
                for nn in range(D // QTILE):
                    nns = slice(nn * QTILE, (nn + 1) * QTILE)
                    po = popool.tile([P, QTILE], f32, tag="po", name="po")
                    for j in range(2 * HEADS_PER_CORE):
                        nc.tensor.matmul(
                            po[:], eT[j // 2][:, j % 2, ts_],
                            ow_sb[j][:, nns],
                            start=(j == 0),
                            stop=(j == 2 * HEADS_PER_CORE - 1),
                        )
                    ob = o3pool.tile([P, QTILE], f32, tag="osb", name="osb")
                    nc.scalar.copy(ob[:], po[:])
                    nc.sync.dma_start(out[ts_, nns], ob[:])

        po_cm.__exit__(None, None, None)
        o3_cm.__exit__(None, None, None)
        psE_cm.__exit__(None, None, None)
        psL_cm.__exit__(None, None, None)
        np_cm.__exit__(None, None, None)
        sp_cm.__exit__(None, None, None)
        et_cm.__exit__(None, None, None)
        owp_cm.__exit__(None, None, None)
        pers_cm.__exit__(None, None, None)

    nc.finalize()
    return nc


def _install_axon_hooks_shim():
    """Provide antenv.axon_hooks if the image lacks it (NTFF profiling)."""
    import types

    try:
        import antenv.axon_hooks  # noqa: F401

        return
    except ImportError:
        pass
    hook = None
    try:
        from trn_agent_boot.trn_boot import _ntff_profile_via_ctypes

        hook = _ntff_profile_via_ctypes("/opt/axon/libaxon_pjrt.so")
    except Exception:
        hook = None
    mod = types.ModuleType("antenv.axon_hooks")
    _h = [hook]
    mod.get_axon_ntff_profile_hook = lambda: _h[0]

    def _set(h):
        _h[0] = h

    mod.set_axon_ntff_profile_hook = _set
    sys.modules["antenv.axon_hooks"] = mod
    try:
        import antenv

        antenv.axon_hooks = mod
    except ImportError:
        pass


def _install_neff_cache():
    """Cache walrus-compiled NEFFs by BIR hash (compiles are minutes-long)."""
    import hashlib
    import shutil

    import concourse.bass2jax as b2j

    if getattr(b2j, "_ant_neff_cache_installed", False):
        return
    orig = b2j.compile_bir_kernel

    def cached(bir_json, tmpdir, neff_name="file.neff"):
        cdir = os.environ.get("NEFF_CACHE_DIR", "/tmp/neff_cache")
        os.makedirs(cdir, exist_ok=True)
        h = hashlib.sha256(bir_json).hexdigest()[:32]
        cpath = os.path.join(cdir, f"{h}.neff")
        if os.path.exists(cpath):
            dst = os.path.join(tmpdir, "sg00")
            os.makedirs(dst, exist_ok=True)
            dstf = os.path.join(dst, neff_name)
            shutil.copyfile(cpath, dstf)
            return dstf
        r = orig(bir_json, tmpdir, neff_name=neff_name)
        try:
            shutil.copyfile(r, cpath)
        except OSError:
            pass
        return r

    b2j.compile_bir_kernel = cached
    b2j._ant_neff_cache_installed = True


def _host_inputs(x, segment_pos, q_w, kv_w, out_w):
    """Per-core input maps (bf16 host-side prep)."""
    import ml_dtypes

    bf = ml_dtypes.bfloat16
    QS = SCALE / SOFT_CAP

    def _wlayout(w):
        # [nh, D, H] -> [nh, P, DC, H]: per-partition contiguous spans
        return np.ascontiguousarray(
            w.reshape(-1, DC, P, w.shape[-1]).transpose(0, 2, 1, 3)
        ).astype(bf)

    ropes = []
    for b in range(B):
        pos = segment_pos[b].astype(np.float32)
        fraction = 2.0 * np.arange(P, dtype=np.float32) / H
        timescale = BASE_FREQ**fraction
        ang = pos[None, :] / timescale[:, None]          # [128, T]
        r = np.stack([np.cos(ang), np.sin(ang)])
        ropes.append(
            np.ascontiguousarray(r.transpose(1, 0, 2)).astype(bf)
        )
    m1 = _make_masks()
    masks = np.ascontiguousarray(
        np.concatenate([m1, m1], axis=2).transpose(1, 0, 2)
    ).astype(bf)

    xTs = []
    for b in range(B):
        xt = np.ascontiguousarray(
            x[b].T.reshape(DC, P, T).transpose(1, 0, 2)
        ).astype(bf)
        xTs.append(xt)

    in_maps = []
    for core in range(8):
        b, g = core // 4, core % 4
        qws = _wlayout(q_w[4 * g : 4 * g + 4] * QS)
        kws = _wlayout(kv_w[0, 2 * g : 2 * g + 2])
        # pack both v heads along H: [P, DC, 2H]
        vss = _wlayout(kv_w[1, 2 * g : 2 * g + 2])   # [2, P, DC, H]
        vwp = np.ascontiguousarray(
            np.concatenate([vss[0], vss[1]], axis=-1)
        )
        ows = np.ascontiguousarray(
            out_w[4 * g : 4 * g + 4].reshape(2 * HEADS_PER_CORE, P, D)
        ).astype(bf)
        in_maps.append(
            {
                "xT": xTs[b],
                "qw": qws,
                "kw": kws,
                "vw": vwp,
                "ow": ows,
                "rope": ropes[b],
                "msk": masks,
            }
        )
    return in_maps


def kernel(x, segment_pos, attn_mask, q_w, kv_w, out_w):
    global LAST_RESULTS
    from concourse.bass_utils import run_bass_kernel_spmd

    _install_axon_hooks_shim()
    _install_neff_cache()

    x = np.asarray(x, np.float32)
    segment_pos = np.asarray(segment_pos, np.int32)
    q_w = np.asarray(q_w, np.float32)
    kv_w = np.asarray(kv_w, np.float32)
    out_w = np.asarray(out_w, np.float32)

    key = "main"
    if key not in _NC_CACHE:
        _NC_CACHE[key] = _build_nc()
    nc = _NC_CACHE[key]

    in_maps = _host_inputs(x, segment_pos, q_w, kv_w, out_w)
    res = run_bass_kernel_spmd(nc, in_maps, core_ids=list(range(8)))
    LAST_RESULTS = res

    outv = np.zeros((B, T, D), np.float32)
    for core in range(8):
        outv[core // 4] += res.results[core]["out"]
    return outv


# revision 6
# speedup vs baseline: 1.1215x; 1.0472x over previous
"""Trainium2 Bass kernel V2: GQA sliding-window attention, SBUF-resident bf16.

Problem: B=2, T=2048, D=3584, N=16 q-heads, K=8 kv-heads, H=256,
sliding window 1024, causal, soft-cap 50, query scale 0.0625, RoPE.

Sharding: 8 cores = 2 (batch) x 4 (head groups); each core: 4 q-heads,
2 kv-heads. Host sums the 4 partial out-projections per batch.

V2 design vs baseline:
  - All weights/intermediates bf16 (halves DMA + SBUF, full PE rate).
  - Zero DRAM round-trips: kT/qT/v/eT live in SBUF end-to-end.
  - V projected directly into [t, h] layout (stationary = x chunk), so
    no PE transposes and no copies.
  - SCALE/SOFT_CAP folded into q_w on host; RoPE drain = ACT copy
    (psum->bf16) + 6 bf16 DVE ops (4x mode) writing kT/qT in place.
  - Attention interleaves the two q-heads of each kv head to keep PE fed
    while the softmax chain (ACT tanh/exp, DVE mask-mul) runs.
  - Out-proj reads eT straight from SBUF, accumulating 8 matmuls in PSUM.
"""

import os
import sys

sys.path.insert(0, "/opt/trn_rl_repo")

import numpy as np

B, T, D = 2, 2048, 3584
NQ, NKV, H = 16, 8, 256
P = 128
DC = D // P                 # 28 contraction chunks
HEADS_PER_CORE = 4
KV_PER_CORE = 2
SOFT_CAP = 50.0
SCALE = 0.0625
WINDOW = 1024
BASE_FREQ = 10000.0
QTILE = 512
NQT = T // QTILE            # 4
ATILE = 256
NAT = T // ATILE            # 8
NKT = T // P                # 16

_NC_CACHE = {}
LAST_RESULTS = None


def _kt_list(at):
    """Valid k-tiles for q-block at (ATILE wide); mask index None = full."""
    Q0 = at * ATILE
    out = []
    for kt in range(NKT):
        K0 = kt * P
        if K0 > Q0 + ATILE - 1:
            continue
        if K0 + P - 1 <= Q0 - WINDOW:
            continue
        rel = K0 - Q0
        if rel >= 0:
            out.append((kt, rel // P))
        else:
            w = Q0 - K0 - WINDOW
            if -ATILE < w <= 0:
                out.append((kt, 2 + (-w) // P))
            else:
                out.append((kt, None))
    return out


def _make_masks():
    m = np.zeros((4, P, ATILE), np.float32)
    i = np.arange(P)[:, None]
    j = np.arange(ATILE)[None, :]
    for r in range(2):           # diag: allowed iff i <= j - rel
        m[r] = np.where(i <= j - r * P, 1.0, 0.0)
    for wi in range(2):          # window: allowed iff i > j - wi*128
        m[2 + wi] = np.where(i > j - wi * P, 1.0, 0.0)
    return m


def _build_nc():
    import concourse.bacc as bacc
    import concourse.mybir as mybir
    import concourse.tile as tile
    from concourse import bass_isa

    f32 = mybir.dt.float32
    bf16 = mybir.dt.bfloat16
    AF = mybir.ActivationFunctionType

    nc = bacc.Bacc()
    xT = nc.dram_tensor("xT", (P, DC, T), bf16, kind="ExternalInput")
    qw = nc.dram_tensor("qw", (HEADS_PER_CORE, P, DC, H), bf16,
                        kind="ExternalInput")
    kw = nc.dram_tensor("kw", (KV_PER_CORE, P, DC, H), bf16,
                        kind="ExternalInput")
    vw = nc.dram_tensor("vw", (P, DC, 2 * H), bf16, kind="ExternalInput")
    ow = nc.dram_tensor("ow", (2 * HEADS_PER_CORE, P, D), bf16,
                        kind="ExternalInput")
    rope = nc.dram_tensor("rope", (P, 2, T), bf16, kind="ExternalInput")
    msk = nc.dram_tensor("msk", (P, 4, 2 * ATILE), bf16,
                         kind="ExternalInput")
    out = nc.dram_tensor("out", (T, D), f32, kind="ExternalOutput")

    with tile.TileContext(nc) as tc:
        pers_cm = tc.tile_pool(name="pers", bufs=1)
        pers = pers_cm.__enter__()

        # Persistent SBUF state
        kT = [pers.tile([P, 2, T], bf16, tag=f"kT{i}", name=f"kT{i}")
              for i in range(KV_PER_CORE)]
        # q stored pair-packed: [hc, at-block, head-in-pair, ATILE] so one
        # 512-wide moving operand covers both heads of a kv pair
        qP = [pers.tile([P, 2, NAT, 2, ATILE], bf16, tag=f"qP{i}",
                        name=f"qP{i}") for i in range(KV_PER_CORE)]
        vA = pers.tile([P, NKT, 2 * H], bf16, tag="vA", name="vA")
        masks_sb = pers.tile([P, 4, 2 * ATILE], bf16, tag="msk",
                             name="masks_sb")
        bias_m50 = pers.tile([P, 1], f32, tag="b50", name="bias_m50")
        nc.vector.memset(bias_m50[:], -SOFT_CAP)
        # warm the GpSimd ucode library during P1 so the first real
        # partition_all_reduce in attention doesn't stall ~10us on I$ load
        gwarm = pers.tile([P, 1], f32, tag="gw", name="gwarm")
        nc.gpsimd.partition_all_reduce(gwarm[:], bias_m50[:], P,
                                       bass_isa.ReduceOp.add)

        # ---------------- P1: projections + RoPE ----------------
        w_cm = tc.tile_pool(name="w", bufs=1)
        wpool = w_cm.__enter__()
        # rope table only lives through P1 — keep it in the P1 pool
        rope_sb = wpool.tile([P, 2, T], bf16, tag="rope", name="rope_sb")
        cos_a = rope_sb[:, 0]
        sin_a = rope_sb[:, 1]
        xt_cm = tc.tile_pool(name="xt", bufs=5)
        xpool = xt_cm.__enter__()
        tmp_cm = tc.tile_pool(name="tmp", bufs=2)
        tpool = tmp_cm.__enter__()
        ps1_cm = tc.tile_pool(name="ps1", bufs=1, space="PSUM")
        ps1 = ps1_cm.__enter__()

        for half in range(2):  # 0: k0,k1 + v(packed)   1: q0..q3
            # DMA issue order matters: every dma_start serializes on the
            # sync engine (~0.6us each), so issue the first-needed chunks
            # first: leading halves of the weights, then the first x tiles,
            # then the weight tails (and rope/masks, needed ~40us in).
            CSPLIT = 14

            if half == 0:
                # interleave weight-chunk and x-tile issue in contraction
                # order (geometric chunks): PE consumes ~380KB per d-group,
                # so arrival order must track the d loop or PE starves
                wk = [wpool.tile([P, DC, H], bf16, tag=f"w{j}",
                                 name=f"wk{j}") for j in range(2)]
                wv = wpool.tile([P, DC, 2 * H], bf16, tag="w2", name="wv")
                xt_pre = []

                def _xt_dma(d4):
                    xt = xpool.tile([P, 4, QTILE], bf16, tag="xt",
                                    name="xt")
                    nc.sync.dma_start(
                        xt[:], xT[:, 4 * d4 : 4 * d4 + 4, 0:QTILE]
                    )
                    xt_pre.append(xt)

                bounds = [0, 2, 6, 14, DC]
                for ci in range(4):
                    cs = slice(bounds[ci], bounds[ci + 1])
                    for j in range(2):
                        nc.sync.dma_start(wk[j][:, cs], kw[j][:, cs])
                    nc.sync.dma_start(wv[:, cs], vw[:, cs])
                    _xt_dma(ci)
                nc.sync.dma_start(rope_sb[:], rope[:])
                nc.sync.dma_start(masks_sb[:], msk[:])
            else:
                wq = [wq0]
                for j in range(1, 4):
                    wt = wpool.tile([P, DC, H], bf16, tag=f"w{j}",
                                    name=f"wq{j}")
                    nc.sync.dma_start(wt[:], qw[j])
                    wq.append(wt)

            for n in range(NQT):
                ns = slice(n * QTILE, (n + 1) * QTILE)
                if half == 0 and n == 2:
                    # qw0 prefetch on its own slot mid-half-0: no WAR gate,
                    # and the x-stream has DMA slack by now
                    wq0 = wpool.tile([P, DC, H], bf16, tag="w4", name="wq0")
                    nc.sync.dma_start(wq0[:], qw[0])
                if half == 0:
                    # k psums: tags t0..t3; v psums (per t-chunk): t4..t7
                    kps = [
                        [ps1.tile([P, QTILE], f32, tag=f"t{2 * j + hc}",
                                  name=f"kps{j}{hc}") for hc in range(2)]
                        for j in range(2)
                    ]
                    vps = [ps1.tile([P, 2 * H], f32, tag=f"t{4 + tci}",
                                    name=f"vps{tci}") for tci in range(4)]
                else:
                    qps = [
                        [ps1.tile([P, QTILE], f32, tag=f"t{2 * j + hc}",
                                  name=f"qps{j}{hc}") for hc in range(2)]
                        for j in range(4)
                    ]
                for dp in range(DC // 4):
                    if half == 0 and n == 0 and dp < 4:
                        xt = xt_pre[dp]
                    else:
                        xt = xpool.tile([P, 4, QTILE], bf16, tag="xt",
                                        name="xt")
                        nc.sync.dma_start(
                            xt[:], xT[:, 4 * dp : 4 * dp + 4, ns]
                        )
                    for u in range(4):
                        d = 4 * dp + u
                        st, sp = (d == 0), (d == DC - 1)
                        if half == 0:
                            for j in range(2):
                                for hc in range(2):
                                    nc.tensor.matmul(
                                        kps[j][hc][:],
                                        wk[j][:, d, hc * P : (hc + 1) * P],
                                        xt[:, u], start=st, stop=sp,
                                    )
                            for tci in range(4):
                                nc.tensor.matmul(
                                    vps[tci][:],
                                    xt[:, u, tci * P : (tci + 1) * P],
                                    wv[:, d, :], start=st, stop=sp,
                                )
                        else:
                            for j in range(4):
                                for hc in range(2):
                                    nc.tensor.matmul(
                                        qps[j][hc][:],
                                        wq[j][:, d, hc * P : (hc + 1) * P],
                                        xt[:, u], start=st, stop=sp,
                                    )
                # drains
                cos_t, sin_t = cos_a[:, ns], sin_a[:, ns]
                if half == 0:
                    for tci in range(4):
                        tg = n * 4 + tci
                        nc.scalar.copy(vA[:, tg, :], vps[tci][:])
                    rope_sets = [(j, kps[j]) for j in range(2)]
                else:
                    rope_sets = [(j, qps[j]) for j in range(4)]
                for j, ps in rope_sets:
                    t0 = tpool.tile([P, QTILE], bf16, tag="t0", name="t0")
                    t1 = tpool.tile([P, QTILE], bf16, tag="t1", name="t1")
                    nc.scalar.copy(t0[:], ps[0][:])
                    nc.vector.tensor_copy(t1[:], ps[1][:])
                    c0 = tpool.tile([P, QTILE], bf16, tag="c0", name="c0")
                    s0 = tpool.tile([P, QTILE], bf16, tag="s0", name="s0")
                    c1 = tpool.tile([P, QTILE], bf16, tag="c1", name="c1")
                    s1 = tpool.tile([P, QTILE], bf16, tag="s1", name="s1")
                    nc.vector.tensor_mul(c0[:], t0[:], cos_t)
                    nc.vector.tensor_mul(s0[:], t0[:], sin_t)
                    nc.vector.tensor_mul(c1[:], t1[:], cos_t)
                    nc.vector.tensor_mul(s1[:], t1[:], sin_t)
                    if half == 0:
                        nc.vector.tensor_sub(kT[j][:, 0, ns], c0[:], s1[:])
                        nc.vector.tensor_add(kT[j][:, 1, ns], c1[:], s0[:])
                    else:
                        pr, ab = j // 2, j % 2
                        for h2 in range(2):
                            cs = slice(h2 * ATILE, (h2 + 1) * ATILE)
                            nc.vector.tensor_sub(
                                qP[pr][:, 0, 2 * n + h2, ab],
                                c0[:, cs], s1[:, cs],
                            )
                            nc.vector.tensor_add(
                                qP[pr][:, 1, 2 * n + h2, ab],
                                c1[:, cs], s0[:, cs],
                            )

        ps1_cm.__exit__(None, None, None)
        tmp_cm.__exit__(None, None, None)
        xt_cm.__exit__(None, None, None)
        w_cm.__exit__(None, None, None)

        # ---------------- P2: attention ----------------
        owp_cm = tc.tile_pool(name="owp", bufs=1)
        owp = owp_cm.__enter__()
        ow_sb = []
        for j in range(2 * HEADS_PER_CORE):
            wt = owp.tile([P, D], bf16, tag=f"owp{j}", name=f"owp{j}")
            nc.sync.dma_start(wt[:], ow[j])
            ow_sb.append(wt)

        et_cm = tc.tile_pool(name="et", bufs=1)
        etp = et_cm.__enter__()
        eT = [etp.tile([P, 2, T], bf16, tag=f"eT{i}", name=f"eT{i}")
              for i in range(HEADS_PER_CORE)]

        sp_cm = tc.tile_pool(name="sp", bufs=3)
        spool = sp_cm.__enter__()
        np_cm = tc.tile_pool(name="np", bufs=3)
        npool = np_cm.__enter__()
        psL_cm = tc.tile_pool(name="psL", bufs=3, space="PSUM")
        psL = psL_cm.__enter__()
        psE_cm = tc.tile_pool(name="psE", bufs=1, space="PSUM")
        psE = psE_cm.__enter__()
        o3_cm = tc.tile_pool(name="o3", bufs=3)
        o3pool = o3_cm.__enter__()
        po_cm = tc.tile_pool(name="po", bufs=3, space="PSUM")
        popool = po_cm.__enter__()

        # attention and out-projection are emitted q-block-major and share
        # PSUM (3 L + 2 enc + 3 out-proj = 8 banks). The two q-heads of
        # each kv head are CONCATENATED along the free dim (qA|qB, 512
        # wide), so QK, tanh, exp, mask-mul, PV, and the denominator chain
        # each run once per k-tile for both heads at full 512-wide rates.
        # Denominator: DVE accumulate + GpSimd partition_all_reduce (no
        # ones-matmul, no PSUM bank, no 1-partition reciprocal).
        W2 = 2 * ATILE

        def emit_p3(a):
            # out-projection for q-block a's two t-chunks
            for tci in range(2 * a, 2 * a + 2):
                ts_ = slice(tci * P, (tci + 1) * P)
                for nn in range(D // QTILE):
                    nns = slice(nn * QTILE, (nn + 1) * QTILE)
                    po = popool.tile([P, QTILE], f32, tag="po", name="po")
                    for j in range(2 * HEADS_PER_CORE):
                        nc.tensor.matmul(
                            po[:], eT[j // 2][:, j % 2, ts_],
                            ow_sb[j][:, nns],
                            start=(j == 0),
                            stop=(j == 2 * HEADS_PER_CORE - 1),
                        )
                    ob = o3pool.tile([P, QTILE], f32, tag="osb", name="osb")
                    if nn % 2 == 0:
                        nc.vector.tensor_copy(ob[:], po[:])
                    else:
                        nc.scalar.copy(ob[:], po[:])
                    nc.sync.dma_start(out[ts_, nns], ob[:])

        def flush_norm(p):
            # reciprocal is emitted one pair-slot late: by now the
            # partition_all_reduce it waits on has finished, so the DVE
            # FIFO is not blocked; the eT muls run on GpSimd so any
            # residual wait lands on the idle engine, not DVE
            a, ecs, dent = p
            rb = npool.tile([P, W2], f32, tag="rb", name="rb")
            nc.vector.reciprocal_approx_fast(rb[:], dent[:])
            pr2, qs2 = a[1], slice(a[0] * ATILE, (a[0] + 1) * ATILE)
            for ab in range(2):
                h = 2 * pr2 + ab
                cs = slice(ab * ATILE, (ab + 1) * ATILE)
                for hc in range(2):
                    nc.vector.tensor_mul(
                        eT[h][:, hc, qs2], ecs[hc][:, cs], rb[:, cs]
                    )

        pending = None
        for at in range(NAT):
            kts = _kt_list(at)
            for pair in range(2):
                kvh = pair
                kTh = kT[kvh]
                enc_ps = [
                    psE.tile([P, W2], f32, tag=f"enc{hc}", name=f"enc{hc}")
                    for hc in range(2)
                ]
                acc = npool.tile([P, W2], f32, tag="acc", name="acc")
                for i, (kt, mi) in enumerate(kts):
                    st, sp = (i == 0), (i == len(kts) - 1)
                    L = psL.tile([P, W2], f32, tag="L", name="L")
                    for hc in range(2):
                        nc.tensor.matmul(
                            L[:], kTh[:, hc, kt * P : (kt + 1) * P],
                            qP[pair][:, hc, at], start=(hc == 0),
                            stop=(hc == 1),
                        )
                    tt = spool.tile([P, W2], f32, tag="tt", name="tt")
                    nc.scalar.activation(tt[:], L[:], AF.Tanh)
                    pp = spool.tile([P, W2], bf16, tag="pp", name="pp")
                    nc.scalar.activation(
                        pp[:], tt[:], AF.Exp, bias=bias_m50[:],
                        scale=SOFT_CAP,
                    )
                    pu = pp[:]
                    if mi is not None:
                        pm = spool.tile([P, W2], bf16, tag="pm", name="pm")
                        nc.vector.tensor_mul(pm[:], pp[:], masks_sb[:, mi])
                        pu = pm[:]
                    if st:
                        nc.vector.tensor_copy(acc[:], pu)
                    else:
                        nc.vector.tensor_add(acc[:], acc[:], pu)
                    base = kvh * H
                    for hc in range(2):
                        nc.tensor.matmul(
                            enc_ps[hc][:],
                            vA[:, kt, base + hc * P : base + (hc + 1) * P],
                            pu, start=st, stop=sp,
                        )
                # evacuate enc PSUM banks promptly (DVE copies) and start
                # the partition_all_reduce; the reciprocal + eT writes are
                # deferred one slot (see flush_norm)
                ecs = []
                for hc in range(2):
                    ec = npool.tile([P, W2], f32, tag=f"ec{hc}", name="ec")
                    nc.vector.tensor_copy(ec[:], enc_ps[hc][:])
                    ecs.append(ec)
                dent = npool.tile([P, W2], f32, tag="dn", name="dent")
                nc.gpsimd.partition_all_reduce(
                    dent[:], acc[:], P, bass_isa.ReduceOp.add
                )
                if pending is not None:
                    flush_norm(pending)
                    if pending[0][1] == 1:
                        emit_p3(pending[0][0])
                pending = ((at, pair), ecs, dent)
        flush_norm(pending)
        emit_p3(NAT - 1)

        po_cm.__exit__(None, None, None)
        o3_cm.__exit__(None, None, None)
        psE_cm.__exit__(None, None, None)
        psL_cm.__exit__(None, None, None)
        np_cm.__exit__(None, None, None)
        sp_cm.__exit__(None, None, None)
        et_cm.__exit__(None, None, None)
        owp_cm.__exit__(None, None, None)
        pers_cm.__exit__(None, None, None)

    nc.finalize()
    return nc


def _install_axon_hooks_shim():
    """Provide antenv.axon_hooks if the image lacks it (NTFF profiling)."""
    import types

    try:
        import antenv.axon_hooks  # noqa: F401

        return
    except ImportError:
        pass
    hook = None
    try:
        from trn_agent_boot.trn_boot import _ntff_profile_via_ctypes

        hook = _ntff_profile_via_ctypes("/opt/axon/libaxon_pjrt.so")
    except Exception:
        hook = None
    mod = types.ModuleType("antenv.axon_hooks")
    _h = [hook]
    mod.get_axon_ntff_profile_hook = lambda: _h[0]

    def _set(h):
        _h[0] = h

    mod.set_axon_ntff_profile_hook = _set
    sys.modules["antenv.axon_hooks"] = mod
    try:
        import antenv

        antenv.axon_hooks = mod
    except ImportError:
        pass


def _install_neff_cache():
    """Cache walrus-compiled NEFFs by BIR hash (compiles are minutes-long)."""
    import hashlib
    import shutil

    import concourse.bass2jax as b2j

    if getattr(b2j, "_ant_neff_cache_installed", False):
        return
    orig = b2j.compile_bir_kernel

    def cached(bir_json, tmpdir, neff_name="file.neff"):
        cdir = os.environ.get("NEFF_CACHE_DIR", "/tmp/neff_cache")
        os.makedirs(cdir, exist_ok=True)
        h = hashlib.sha256(bir_json).hexdigest()[:32]
        cpath = os.path.join(cdir, f"{h}.neff")
        if os.path.exists(cpath):
            dst = os.path.join(tmpdir, "sg00")
            os.makedirs(dst, exist_ok=True)
            dstf = os.path.join(dst, neff_name)
            shutil.copyfile(cpath, dstf)
            return dstf
        r = orig(bir_json, tmpdir, neff_name=neff_name)
        try:
            shutil.copyfile(r, cpath)
        except OSError:
            pass
        return r

    b2j.compile_bir_kernel = cached
    b2j._ant_neff_cache_installed = True


def _host_inputs(x, segment_pos, q_w, kv_w, out_w):
    """Per-core input maps (bf16 host-side prep)."""
    import ml_dtypes

    bf = ml_dtypes.bfloat16
    QS = SCALE / SOFT_CAP

    def _wlayout(w):
        # [nh, D, H] -> [nh, P, DC, H]: per-partition contiguous spans
        return np.ascontiguousarray(
            w.reshape(-1, DC, P, w.shape[-1]).transpose(0, 2, 1, 3)
        ).astype(bf)

    ropes = []
    for b in range(B):
        pos = segment_pos[b].astype(np.float32)
        fraction = 2.0 * np.arange(P, dtype=np.float32) / H
        timescale = BASE_FREQ**fraction
        ang = pos[None, :] / timescale[:, None]          # [128, T]
        r = np.stack([np.cos(ang), np.sin(ang)])
        ropes.append(
            np.ascontiguousarray(r.transpose(1, 0, 2)).astype(bf)
        )
    m1 = _make_masks()
    masks = np.ascontiguousarray(
        np.concatenate([m1, m1], axis=2).transpose(1, 0, 2)
    ).astype(bf)

    xTs = []
    for b in range(B):
        xt = np.ascontiguousarray(
            x[b].T.reshape(DC, P, T).transpose(1, 0, 2)
        ).astype(bf)
        xTs.append(xt)

    in_maps = []
    for core in range(8):
        b, g = core // 4, core % 4
        qws = _wlayout(q_w[4 * g : 4 * g + 4] * QS)
        kws = _wlayout(kv_w[0, 2 * g : 2 * g + 2])
        # pack both v heads along H: [P, DC, 2H]
        vss = _wlayout(kv_w[1, 2 * g : 2 * g + 2])   # [2, P, DC, H]
        vwp = np.ascontiguousarray(
            np.concatenate([vss[0], vss[1]], axis=-1)
        )
        ows = np.ascontiguousarray(
            out_w[4 * g : 4 * g + 4].reshape(2 * HEADS_PER_CORE, P, D)
        ).astype(bf)
        in_maps.append(
            {
                "xT": xTs[b],
                "qw": qws,
                "kw": kws,
                "vw": vwp,
                "ow": ows,
                "rope": ropes[b],
                "msk": masks,
            }
        )
    return in_maps


def kernel(x, segment_pos, attn_mask, q_w, kv_w, out_w):
    global LAST_RESULTS
    from concourse.bass_utils import run_bass_kernel_spmd

    _install_axon_hooks_shim()
    _install_neff_cache()

    x = np.asarray(x, np.float32)
    segment_pos = np.asarray(segment_pos, np.int32)
    q_w = np.asarray(q_w, np.float32)
    kv_w = np.asarray(kv_w, np.float32)
    out_w = np.asarray(out_w, np.float32)

    key = "main"
    if key not in _NC_CACHE:
        _NC_CACHE[key] = _build_nc()
    nc = _NC_CACHE[key]

    in_maps = _host_inputs(x, segment_pos, q_w, kv_w, out_w)
    res = run_bass_kernel_spmd(nc, in_maps, core_ids=list(range(8)))
    LAST_RESULTS = res

    outv = np.zeros((B, T, D), np.float32)
    for core in range(8):
        outv[core // 4] += res.results[core]["out"]
    return outv
